# revision 1
# baseline (speedup 1.0000x reference)
"""ArcFace head on 8 TRN2 NeuronCores (Bass/Tile).

Model-parallel over classes: each of the 8 cores owns a 12500-class slice
of the 100000-class weight matrix and computes its (1024 x 12500) slice of
the logits; the host concatenates slices along the class dim.

Per-core device kernel:
  - normalize embeddings (64/||e|| folded in) and the weight slice
    (1/||w|| folded in), cast to bf16
  - (1024 x 512) @ (512 x 12500) matmul on TensorE, f32 accumulate
  - ArcFace margin: gather the label's weight row per sample
    (indirect DMA), compute cos(theta+m) per row in f32, scatter the
    corrected target logits into the output (indirect DMA, out-of-shard
    rows skipped via the bounds check)

Inputs are pre-arranged on the host (transposed weight slice for the
matmul operands, per-shard relabeled indices); all arithmetic of the op
itself runs on device.
"""

import math

import ml_dtypes
import numpy as np

import concourse.bacc as bacc
import concourse.bass as bass
import concourse.mybir as mybir
import concourse.tile as tile

# Problem constants (hardcoded per harness rules).
B = 1024  # batch
D = 512  # embedding dim
C = 100000  # num classes
NCORES = 8
CS = C // NCORES  # classes per core = 12500
P = 128  # partitions
KCH = D // P  # contraction chunks = 4
NB = B // P  # batch tiles = 8
CW = 500  # class window per matmul (<=512 psum bank, divides 12500)
NCW = CS // CW  # 25 class windows

SCALE = 64.0
MARGIN = 0.5
COS_M = math.cos(MARGIN)
SIN_M = math.sin(MARGIN)
TH = math.cos(math.pi - MARGIN)
MM = math.sin(math.pi - MARGIN) * MARGIN

F32 = mybir.dt.float32
BF16 = mybir.dt.bfloat16
I32 = mybir.dt.int32

OOB_SCATTER = 1 << 26  # out-of-shard sentinel for scatter offsets


def build_graph():
    nc = bacc.Bacc(
        "TRN2",
        target_bir_lowering=False,
        debug=False,
        num_devices=NCORES,
    )

    embT = nc.declare_dram_parameter("embT", [D, B], F32, isOutput=False)
    wT = nc.declare_dram_parameter("wT", [D, CS], BF16, isOutput=False)
    w_nat = nc.declare_dram_parameter("w_nat", [CS, D], F32, isOutput=False)
    emb = nc.declare_dram_parameter("emb", [B, D], F32, isOutput=False)
    gidx = nc.declare_dram_parameter("gidx", [P, NB], I32, isOutput=False)
    soff = nc.declare_dram_parameter("soff", [P, NB], I32, isOutput=False)
    out = nc.declare_dram_parameter("out", [B, CS], BF16, isOutput=True)

    # DRAM views: partition p of contraction chunk k holds row k*128+p;
    # batch row b maps to (partition b%128, tile b//128).
    embT_r = embT[:].rearrange("(k p) b -> p k b", p=P)  # (128, 4, 1024)
    wT_r = wT[:].rearrange("(k p) c -> p k c", p=P)  # (128, 4, 12500)
    emb_r = emb[:].rearrange("(i p) d -> p i d", p=P)  # (128, 8, 512)
    out_r = out[:].rearrange("(i p) c -> p i c", p=P)  # (128, 8, 12500)
    out_flat = out[:].rearrange("a b -> (a b)")[:, None]  # (12.8M, 1)

    with tile.TileContext(nc) as tc:
        with (
            tc.tile_pool(name="const", bufs=1) as constp,
            tc.tile_pool(name="embp", bufs=1) as embp,
            tc.tile_pool(name="wstage", bufs=6) as wstage,
            tc.tile_pool(name="wnb", bufs=2) as wnbp,
            tc.tile_pool(name="wsq", bufs=2) as wsqp,
            tc.tile_pool(name="wnt", bufs=5) as wntp,
            tc.tile_pool(name="ostripe", bufs=4) as ostripep,
            tc.tile_pool(name="small", bufs=2) as smallp,
            tc.tile_pool(name="marg", bufs=1) as margp,
            tc.tile_pool(name="ps_main", bufs=3, space="PSUM") as ps_main,
            tc.tile_pool(name="ps_small", bufs=2, space="PSUM") as ps_small,
        ):
            # Constants.
            ones_col_bf = constp.tile([P, 1], BF16, tag="ones_col")
            nc.vector.memset(ones_col_bf[:], 1.0)

            # ---------- weight prep, software-pipelined two iterations ahead
            # of the matmuls that consume it (w DMAs lead the input queue)
            def prep(cw):
                csl = slice(cw * CW, (cw + 1) * CW)
                wt_f = wstage.tile([P, KCH, CW], BF16, tag="wt_f")
                nc.sync.dma_start(out=wt_f[:], in_=wT_r[:, :, csl])
                w2 = wsqp.tile([P, KCH, CW], BF16, tag="w2")
                nc.scalar.square(w2[:], wt_f[:])
                # fold 4 contraction chunks to 2 on VectorE so the norm
                # reduction costs TensorE two matmuls instead of four
                w2s = wsqp.tile([P, 2, CW], BF16, tag="w2s")
                nc.vector.tensor_add(w2s[:, 0, :], w2[:, 0, :], w2[:, 1, :])
                nc.vector.tensor_add(w2s[:, 1, :], w2[:, 2, :], w2[:, 3, :])
                pn = ps_small.tile([1, 512], F32, tag="ps_small")
                for k in range(2):
                    nc.tensor.matmul(
                        pn[:, :CW],
                        lhsT=ones_col_bf[:],
                        rhs=w2s[:, k, :],
                        start=(k == 0),
                        stop=(k == 1),
                    )
                rn = smallp.tile([1, CW], F32, tag="rn")
                nc.scalar.sqrt(rn[:], pn[:, :CW])
                rrec = smallp.tile([1, CW], F32, tag="rrec")
                rscrw = smallp.tile([1, CW], F32, tag="rscrw")
                nc.vector.reciprocal_approx_accurate(rrec[:], rn[:], rscrw[:])
                rrecb = smallp.tile([1, CW], BF16, tag="rrecb")
                nc.scalar.copy(rrecb[:], rrec[:])
                wnb = wnbp.tile([P, CW], BF16, tag="wnb")
                nc.gpsimd.partition_broadcast(wnb[:], rrecb[:])
                wnt = wntp.tile([P, KCH, CW], BF16, tag="wnt")
                nc.vector.tensor_mul(
                    wnt[:],
                    wt_f[:],
                    wnb[:, None, :].to_broadcast([P, KCH, CW]),
                )
                return wnt

            wnt_q = [prep(0), prep(1)]

            # ---------- embedding prep: embT_n = 64 * emb.T / ||emb|| (bf16)
            # processed in two batch halves so the first half unblocks the
            # main matmuls as early as possible
            embT_f = embp.tile([P, KCH, B], F32, tag="embT_f")
            emb2 = embp.tile([P, KCH, B], BF16, tag="emb2")
            ebb = embp.tile([P, B], F32, tag="ebb")
            embT_n = embp.tile([P, KCH, B], BF16, tag="embT_n")
            enorm = smallp.tile([1, B], F32, tag="enorm")
            erec = smallp.tile([1, B], F32, tag="erec")
            escr = smallp.tile([1, B], F32, tag="escr")
            for h in range(2):
                hs = slice(h * 512, (h + 1) * 512)
                nc.sync.dma_start(out=embT_f[:, :, hs], in_=embT_r[:, :, hs])
                nc.scalar.square(emb2[:, :, hs], embT_f[:, :, hs])
                pe = ps_small.tile([1, 512], F32, tag="ps_small")
                for k in range(KCH):
                    nc.tensor.matmul(
                        pe[:],
                        lhsT=ones_col_bf[:],
                        rhs=emb2[:, k, hs],
                        start=(k == 0),
                        stop=(k == KCH - 1),
                    )
                nc.scalar.sqrt(enorm[:, hs], pe[:])
                nc.vector.reciprocal_approx_accurate(
                    erec[:, hs], enorm[:, hs], escr[:, hs]
                )
                nc.vector.tensor_scalar_mul(erec[:, hs], erec[:, hs], SCALE)
                nc.gpsimd.partition_broadcast(ebb[:, hs], erec[:, hs])
                for k in range(KCH):
                    nc.vector.tensor_mul(
                        embT_n[:, k, hs], embT_f[:, k, hs], ebb[:, hs]
                    )

            # ---------- margin path: corrected target logits per sample.
            # Data movement happens up front (sync/gpsimd queues); the
            # vector/scalar compute is emitted interleaved with the main
            # loop (one op per class window) so it never delays the
            # weight-prep chain feeding TensorE.
            emb_nat = margp.tile([P, NB, D], F32, tag="emb_nat")
            nc.scalar.dma_start(out=emb_nat[:], in_=emb_r[:])
            gidx_t = margp.tile([P, NB], I32, tag="gidx_t")
            nc.scalar.dma_start(out=gidx_t[:], in_=gidx[:])
            soff_t = margp.tile([P, NB], I32, tag="soff_t")
            nc.scalar.dma_start(out=soff_t[:], in_=soff[:])

            wg = margp.tile([P, NB, D], F32, tag="wg")
            nc.gpsimd.memset(wg[:], 0.0)
            # gather w rows for in-shard labels; out-of-shard rows skipped
            # (one offset per partition per call — the layout the HW
            # indirect DGE path supports)
            for i in range(NB):
                nc.gpsimd.indirect_dma_start(
                    out=wg[:, i, :],
                    out_offset=None,
                    in_=w_nat[:],
                    in_offset=bass.IndirectOffsetOnAxis(
                        ap=gidx_t[:, i : i + 1], axis=0
                    ),
                    bounds_check=CS - 1,
                    oob_is_err=False,
                )

            mtmp = margp.tile([P, D], F32, tag="mtmp")
            en2 = margp.tile([P, NB], F32, tag="en2")
            gn2 = margp.tile([P, NB], F32, tag="gn2")
            dot = margp.tile([P, NB], F32, tag="dot")
            den = margp.tile([P, NB], F32, tag="den")
            rden = margp.tile([P, NB], F32, tag="rden")
            rscr = margp.tile([P, NB], F32, tag="rscr")
            cost = margp.tile([P, NB], F32, tag="cost")
            sint = margp.tile([P, NB], F32, tag="sint")
            cosm = margp.tile([P, NB], F32, tag="cosm")
            alt = margp.tile([P, NB], F32, tag="alt")
            mask = margp.tile([P, NB], mybir.dt.uint8, tag="mask")
            yv = margp.tile([P, NB], F32, tag="yv")
            yvb = margp.tile([P, NB], BF16, tag="yvb")
            X = mybir.AxisListType.X
            ADD = mybir.AluOpType.add

            def rowdot(a, b, acc, i):
                # acc[:, i] = sum_d a[:, i, :] * b[:, i, :], as two small ops
                def mul():
                    nc.vector.tensor_mul(mtmp[:], a[:, i, :], b[:, i, :])

                def red():
                    nc.vector.tensor_reduce(
                        acc[:, i : i + 1], mtmp[:, None, :], axis=X, op=ADD
                    )

                return [mul, red]

            margin_ops = []
            for a, b, acc in (
                (emb_nat, emb_nat, en2),
                (wg, wg, gn2),
                (emb_nat, wg, dot),
            ):
                for i in range(NB):
                    margin_ops += rowdot(a, b, acc, i)
            margin_ops += [
                # cos_t = dot / max(||e||*||w_label||, eps)
                lambda: nc.vector.tensor_mul(den[:], en2[:], gn2[:]),
                lambda: nc.scalar.sqrt(den[:], den[:]),
                lambda: nc.vector.tensor_scalar_max(den[:], den[:], 1e-12),
                lambda: nc.vector.reciprocal_approx_accurate(
                    rden[:], den[:], rscr[:]
                ),
                lambda: nc.vector.tensor_mul(cost[:], dot[:], rden[:]),
                # sin_t = sqrt(max(0, 1 - cos^2))
                lambda: nc.vector.tensor_mul(sint[:], cost[:], cost[:]),
                lambda: nc.vector.tensor_scalar(
                    out=sint[:],
                    in0=sint[:],
                    scalar1=-1.0,
                    scalar2=1.0,
                    op0=mybir.AluOpType.mult,
                    op1=ADD,
                ),
                lambda: nc.vector.tensor_scalar_max(sint[:], sint[:], 0.0),
                lambda: nc.scalar.sqrt(sint[:], sint[:]),
                # cos(t+m) = cos*COS_M - sin*SIN_M ; else branch: cos - MM
                lambda: nc.vector.tensor_scalar_mul(cosm[:], sint[:], -SIN_M),
                lambda: nc.vector.scalar_tensor_tensor(
                    out=cosm[:],
                    in0=cost[:],
                    scalar=COS_M,
                    in1=cosm[:],
                    op0=mybir.AluOpType.mult,
                    op1=ADD,
                ),
                lambda: nc.vector.tensor_scalar_add(alt[:], cost[:], -MM),
                lambda: nc.vector.tensor_single_scalar(
                    mask[:], cost[:], TH, mybir.AluOpType.is_gt
                ),
                lambda: nc.vector.select(yv[:], mask[:], cosm[:], alt[:]),
                lambda: nc.vector.tensor_scalar_mul(yv[:], yv[:], SCALE),
                lambda: nc.vector.tensor_copy(yvb[:], yv[:]),
            ]


            for cw in range(NCW):
                if cw + 2 < NCW:
                    wnt_q.append(prep(cw + 2))
                wnt_cur = wnt_q.pop(0)
                ostripe = ostripep.tile([P, NB, CW], BF16, tag="ostripe")
                for half in range(NB // 2):
                    # pair of bank-aligned psum tiles drained in one op
                    po2 = ps_main.tile([P, 2, 512], F32, tag="ps_main")
                    for j in range(2):
                        bt = half * 2 + j
                        for k in range(KCH):
                            nc.tensor.matmul(
                                po2[:, j, :CW],
                                lhsT=embT_n[:, k, bt * P : (bt + 1) * P],
                                rhs=wnt_cur[:, k, :],
                                start=(k == 0),
                                stop=(k == KCH - 1),
                            )
                    if half % 2 == 0:
                        nc.scalar.copy(
                            ostripe[:, half * 2 : half * 2 + 2, :], po2[:, :, :CW]
                        )
                    else:
                        nc.vector.tensor_copy(
                            ostripe[:, half * 2 : half * 2 + 2, :], po2[:, :, :CW]
                        )
                # out-DMAs on the gpsimd (SWDGE) queue so they never block
                # the sync queue's input prefetch stream
                nc.gpsimd.dma_start(
                    out=out_r[:, :, cw * CW : (cw + 1) * CW], in_=ostripe[:]
                )
                for _ in range(3):
                    if margin_ops:
                        margin_ops.pop(0)()
            while margin_ops:
                margin_ops.pop(0)()

            # ---------- scatter corrected target logits (after main writes)
            for i in range(NB):
                nc.gpsimd.indirect_dma_start(
                    out=out_flat,
                    out_offset=bass.IndirectOffsetOnAxis(
                        ap=soff_t[:, i : i + 1], axis=0
                    ),
                    in_=yvb[:, i : i + 1],
                    in_offset=None,
                    bounds_check=B * CS - 1,
                    oob_is_err=False,
                )

    nc.compile()
    return nc


def make_in_maps(embeddings, labels, weight):
    """Shard + lay out the inputs for the 8 cores."""
    emb = np.ascontiguousarray(embeddings, dtype=np.float32)
    embT = np.ascontiguousarray(emb.T)
    lab = np.asarray(labels).astype(np.int64)
    w = np.asarray(weight, dtype=np.float32)

    bidx = np.arange(B)
    p_of_b = bidx % P  # partition
    i_of_b = bidx // P  # batch tile

    in_maps = []
    for c in range(NCORES):
        lo = c * CS
        local = lab - lo
        in_shard = (local >= 0) & (local < CS)
        gidx = np.full((P, NB), CS, dtype=np.int32)  # CS -> OOB, skipped
        gidx[p_of_b, i_of_b] = np.where(in_shard, local, CS).astype(np.int32)
        soff = np.full((P, NB), OOB_SCATTER, dtype=np.int32)
        soff[p_of_b, i_of_b] = np.where(
            in_shard, bidx * CS + np.clip(local, 0, CS - 1), OOB_SCATTER
        ).astype(np.int32)
        wsh = w[lo : lo + CS]
        in_maps.append(
            {
                "embT": embT,
                "wT": np.ascontiguousarray(wsh.T).astype(ml_dtypes.bfloat16),
                "w_nat": np.ascontiguousarray(wsh),
                "emb": emb,
                "gidx": gidx,
                "soff": soff,
            }
        )
    return in_maps


_CACHED_NC = None


def _get_graph():
    global _CACHED_NC
    if _CACHED_NC is None:
        _CACHED_NC = build_graph()
    return _CACHED_NC


def kernel(embeddings, labels, weight):
    from concourse.bass_utils import run_bass_kernel_spmd

    nc = _get_graph()
    in_maps = make_in_maps(embeddings, labels, weight)
    res = run_bass_kernel_spmd(nc, in_maps, core_ids=list(range(NCORES)))
    return np.concatenate(
        [res.results[i]["out"].astype(np.float32) for i in range(NCORES)], axis=1
    )


if __name__ == "__main__":
    nc = build_graph()
    print("graph built ok")



# revision 3
# speedup vs baseline: 1.0077x; 1.0077x over previous
"""ArcFace head on 8 TRN2 NeuronCores (Bass/Tile).

Model-parallel over classes: each of the 8 cores owns a 12500-class slice
of the 100000-class weight matrix and computes its (1024 x 12500) slice of
the logits; the host reassembles slices along the class dim.

v2 design (vs v1): raw bf16 weights feed TensorE straight from DMA; the
1/||w_c|| and 64/||e_b|| normalizations are folded into the PSUM->SBUF
drain (scalar_tensor_tensor: psum * ebn[p] * wnb). Weight norms are
computed per class window from squares folded on DVE and reduced across
partitions (gpsimd partition_all_reduce or a ones-column matmul). The
ArcFace margin is computed early from gathered label rows (exact f32 cos)
into a tiny tfix output that the host overlays onto the final array - no
scatter into the big output, so no ordering tail. DRAM layouts are
window-major so every DMA moves contiguous multi-KB runs per partition.
"""

import math

import ml_dtypes
import numpy as np

import concourse.bacc as bacc
import concourse.bass as bass
import concourse.bass_isa as bass_isa
import concourse.mybir as mybir
import concourse.tile as tile

# Problem constants (hardcoded per harness rules).
B = 1024  # batch
D = 512  # embedding dim
C = 100000  # num classes
NCORES = 8
CS = C // NCORES  # classes per core = 12500
P = 128  # partitions
KCH = D // P  # contraction chunks = 4
NB = B // P  # batch tiles = 8
CW = 500  # class window (<=512 psum bank, divides 12500)
NCW = CS // CW  # 25 class windows

SCALE = 64.0
MARGIN = 0.5
COS_M = math.cos(MARGIN)
SIN_M = math.sin(MARGIN)
TH = math.cos(math.pi - MARGIN)
MM = math.sin(math.pi - MARGIN) * MARGIN

F32 = mybir.dt.float32
BF16 = mybir.dt.bfloat16
I32 = mybir.dt.int32
U8 = mybir.dt.uint8

NORM_MODE = "gpsimd"  # "gpsimd" (partition_all_reduce) or "mm" (ones matmul)


def build_graph():
    nc = bacc.Bacc(
        "TRN2",
        target_bir_lowering=False,
        debug=False,
        num_devices=NCORES,
    )

    embT_l = nc.declare_dram_parameter("embT_l", [P, KCH, B], BF16, isOutput=False)
    wt_l = nc.declare_dram_parameter("wt_l", [NCW, P, KCH, CW], BF16, isOutput=False)
    emb_n = nc.declare_dram_parameter("emb_n", [P, NB, D], BF16, isOutput=False)
    w_nat = nc.declare_dram_parameter("w_nat", [CS, D], BF16, isOutput=False)
    gidx = nc.declare_dram_parameter("gidx", [P, NB], I32, isOutput=False)
    out_dev = nc.declare_dram_parameter(
        "out_dev", [NCW, P, NB, CW], BF16, isOutput=True
    )
    tfix = nc.declare_dram_parameter("tfix", [P, NB], BF16, isOutput=True)

    ADD = mybir.AluOpType.add
    MUL = mybir.AluOpType.mult

    with tile.TileContext(nc) as tc:
        with (
            tc.tile_pool(name="const", bufs=1) as constp,
            tc.tile_pool(name="embp", bufs=1) as embp,
            tc.tile_pool(name="wstage", bufs=4) as wstage,
            tc.tile_pool(name="w2p", bufs=2) as w2p,
            tc.tile_pool(name="wnbp", bufs=3) as wnbp,
            tc.tile_pool(name="nsqp", bufs=2) as nsqp,
            tc.tile_pool(name="ostripe", bufs=3) as ostripep,
            tc.tile_pool(name="marg", bufs=1) as margp,
            tc.tile_pool(name="ps_main", bufs=3, space="PSUM") as ps_main,
            tc.tile_pool(name="ps_small", bufs=1, space="PSUM") as ps_small,
        ):
            if NORM_MODE == "mm":
                ones_col_bf = constp.tile([P, 1], BF16, tag="ones_col")
                nc.vector.memset(ones_col_bf[:], 1.0)

            # ---------- stationary inputs
            embT_t = embp.tile([P, KCH, B], BF16, tag="embT_t")
            nc.sync.dma_start(out=embT_t[:], in_=embT_l[:])
            emb_t = margp.tile([P, NB, D], BF16, tag="emb_t")
            nc.scalar.dma_start(out=emb_t[:], in_=emb_n[:])
            gidx_t = margp.tile([P, NB], I32, tag="gidx_t")
            nc.scalar.dma_start(out=gidx_t[:], in_=gidx[:])

            # ---------- per-window weight prep: wt DMA + 1/||w_c|| (wnb)
            def prep(cw):
                wt_f = wstage.tile([P, KCH, CW], BF16, tag="wt_f")
                nc.sync.dma_start(out=wt_f[:], in_=wt_l[cw])
                w2 = w2p.tile([P, KCH, CW], BF16, tag="w2")
                nc.scalar.square(w2[:], wt_f[:])
                w2b = w2p.tile([P, 2, CW], BF16, tag="w2b")
                nc.vector.tensor_add(w2b[:, 0, :], w2[:, 0, :], w2[:, 1, :])
                nc.vector.tensor_add(w2b[:, 1, :], w2[:, 2, :], w2[:, 3, :])
                wnb = wnbp.tile([P, CW], F32, tag="wnb")
                if NORM_MODE == "gpsimd":
                    w2s = w2p.tile([P, CW], BF16, tag="w2s")
                    nc.vector.tensor_add(w2s[:], w2b[:, 0, :], w2b[:, 1, :])
                    nsq = nsqp.tile([P, CW], F32, tag="nsq")
                    nc.gpsimd.partition_all_reduce(
                        nsq[:], w2s[:], channels=P, reduce_op=bass_isa.ReduceOp.add
                    )
                    nrt = nsqp.tile([P, CW], F32, tag="nrt")
                    nc.scalar.sqrt(nrt[:], nsq[:])
                    nc.vector.reciprocal_approx_fast(wnb[:], nrt[:])
                else:
                    pn = ps_small.tile([1, 512], F32, tag="pn")
                    for k in range(2):
                        nc.tensor.matmul(
                            pn[:, :CW],
                            lhsT=ones_col_bf[:],
                            rhs=w2b[:, k, :],
                            start=(k == 0),
                            stop=(k == 1),
                        )
                    rn = nsqp.tile([1, CW], F32, tag="rn")
                    nc.scalar.sqrt(rn[:], pn[:, :CW])
                    rrec = nsqp.tile([1, CW], F32, tag="rrec")
                    nc.vector.reciprocal_approx_fast(rrec[:], rn[:])
                    nc.gpsimd.partition_broadcast(wnb[:], rrec[:])
                return wt_f, wnb

            win_q = [prep(0), prep(1)]

            # ---------- embedding norms: ebn[p, i] = 64 / ||e_b||, b = i*128+p
            junk_e = margp.tile([P, D], BF16, tag="junk_e")
            en2 = margp.tile([P, NB], F32, tag="en2")
            for i in range(NB):
                nc.scalar.activation(
                    junk_e[:],
                    emb_t[:, i, :],
                    mybir.ActivationFunctionType.Square,
                    accum_out=en2[:, i : i + 1],
                )
            en_s = margp.tile([P, NB], F32, tag="en_s")
            nc.scalar.sqrt(en_s[:], en2[:])
            ebn_r = margp.tile([P, NB], F32, tag="ebn_r")
            ebn_scr = margp.tile([P, NB], F32, tag="ebn_scr")
            nc.vector.reciprocal_approx_accurate(ebn_r[:], en_s[:], ebn_scr[:])
            ebn = margp.tile([P, NB], F32, tag="ebn")
            nc.vector.tensor_scalar_mul(ebn[:], ebn_r[:], SCALE)

            # ---------- margin thunks (drained 2 per window inside the loop):
            # exact f32 target cos from gathered label rows -> tfix output;
            # the host overlays tfix, so nothing waits on the big output.
            wg = margp.tile([P, NB, D], BF16, tag="wg")
            junk_g = margp.tile([P, D], BF16, tag="junk_g")
            junk_d = margp.tile([P, D], BF16, tag="junk_d")
            gn2 = margp.tile([P, NB], F32, tag="gn2")
            dot = margp.tile([P, NB], F32, tag="dot")
            den = margp.tile([P, NB], F32, tag="den")
            rden = margp.tile([P, NB], F32, tag="rden")
            rscr = margp.tile([P, NB], F32, tag="rscr")
            cost = margp.tile([P, NB], F32, tag="cost")
            sint = margp.tile([P, NB], F32, tag="sint")
            cosm = margp.tile([P, NB], F32, tag="cosm")
            alt = margp.tile([P, NB], F32, tag="alt")
            mask = margp.tile([P, NB], U8, tag="mask")
            yv = margp.tile([P, NB], F32, tag="yv")
            tfix_t = margp.tile([P, NB], BF16, tag="tfix_t")

            margin_ops = []
            for i in range(NB):
                margin_ops.append(
                    lambda i=i: nc.gpsimd.indirect_dma_start(
                        out=wg[:, i, :],
                        out_offset=None,
                        in_=w_nat[:],
                        in_offset=bass.IndirectOffsetOnAxis(
                            ap=gidx_t[:, i : i + 1], axis=0
                        ),
                        bounds_check=CS - 1,
                        oob_is_err=False,
                    )
                )
            for i in range(NB):
                margin_ops.append(
                    lambda i=i: nc.scalar.activation(
                        junk_g[:],
                        wg[:, i, :],
                        mybir.ActivationFunctionType.Square,
                        accum_out=gn2[:, i : i + 1],
                    )
                )
            for i in range(NB):
                margin_ops.append(
                    lambda i=i: nc.vector.scalar_tensor_tensor(
                        out=junk_d[:],
                        in0=emb_t[:, i, :],
                        scalar=1.0,
                        in1=wg[:, i, :],
                        op0=MUL,
                        op1=MUL,
                        accum_out=dot[:, i : i + 1],
                    )
                )
            margin_ops += [
                # cos = dot / max(||e|| * ||w_l||, eps)
                lambda: nc.vector.tensor_mul(den[:], en2[:], gn2[:]),
                lambda: nc.scalar.sqrt(den[:], den[:]),
                lambda: nc.vector.tensor_scalar_max(den[:], den[:], 1e-12),
                lambda: nc.vector.reciprocal_approx_accurate(
                    rden[:], den[:], rscr[:]
                ),
                lambda: nc.vector.tensor_mul(cost[:], dot[:], rden[:]),
                # clip to +-(1 - 1e-7)
                lambda: nc.vector.tensor_scalar(
                    out=cost[:],
                    in0=cost[:],
                    scalar1=1.0 - 1e-7,
                    scalar2=-(1.0 - 1e-7),
                    op0=mybir.AluOpType.min,
                    op1=mybir.AluOpType.max,
                ),
                # sin = sqrt(max(0, 1 - cos^2))
                lambda: nc.vector.tensor_mul(sint[:], cost[:], cost[:]),
                lambda: nc.vector.tensor_scalar(
                    out=sint[:],
                    in0=sint[:],
                    scalar1=-1.0,
                    scalar2=1.0,
                    op0=MUL,
                    op1=ADD,
                ),
                lambda: nc.vector.tensor_scalar_max(sint[:], sint[:], 0.0),
                lambda: nc.scalar.sqrt(sint[:], sint[:]),
                # cos(t+m) = cos*COS_M - sin*SIN_M ; else: cos - MM
                lambda: nc.vector.tensor_scalar_mul(cosm[:], sint[:], -SIN_M),
                lambda: nc.vector.scalar_tensor_tensor(
                    out=cosm[:],
                    in0=cost[:],
                    scalar=COS_M,
                    in1=cosm[:],
                    op0=MUL,
                    op1=ADD,
                ),
                lambda: nc.vector.tensor_scalar_add(alt[:], cost[:], -MM),
                lambda: nc.vector.tensor_single_scalar(
                    mask[:], cost[:], TH, mybir.AluOpType.is_gt
                ),
                lambda: nc.vector.select(yv[:], mask[:], cosm[:], alt[:]),
                lambda: nc.vector.tensor_scalar_mul(tfix_t[:], yv[:], SCALE),
                lambda: nc.scalar.dma_start(out=tfix[:], in_=tfix_t[:]),
            ]

            # ---------- main loop: 25 windows x (32 matmuls + 8 drains + DMA)
            for cw in range(NCW):
                if cw + 2 < NCW:
                    win_q.append(prep(cw + 2))
                wt_cur, wnb_cur = win_q.pop(0)
                ostripe = ostripep.tile([P, NB, CW], BF16, tag="ostripe")
                for half in range(NB // 2):
                    po2 = ps_main.tile([P, 2, 512], F32, tag="ps_main")
                    for j in range(2):
                        bt = half * 2 + j
                        for k in range(KCH):
                            nc.tensor.matmul(
                                po2[:, j, :CW],
                                lhsT=embT_t[:, k, bt * P : (bt + 1) * P],
                                rhs=wt_cur[:, k, :],
                                start=(k == 0),
                                stop=(k == KCH - 1),
                            )
                    for j in range(2):
                        bt = half * 2 + j
                        nc.vector.scalar_tensor_tensor(
                            out=ostripe[:, bt, :],
                            in0=po2[:, j, :CW],
                            scalar=ebn[:, bt : bt + 1],
                            in1=wnb_cur[:],
                            op0=MUL,
                            op1=MUL,
                        )
                nc.gpsimd.dma_start(out=out_dev[cw], in_=ostripe[:])
                for _ in range(2):
                    if margin_ops:
                        margin_ops.pop(0)()
            while margin_ops:
                margin_ops.pop(0)()

    nc.compile()
    return nc


def make_in_maps(embeddings, labels, weight):
    """Shard + lay out the inputs for the 8 cores (host-side layout prep)."""
    emb = np.ascontiguousarray(embeddings, dtype=np.float32)
    lab = np.asarray(labels).astype(np.int64)
    w = np.asarray(weight, dtype=np.float32)

    bf16 = ml_dtypes.bfloat16
    # embT_l[p, k, b] = emb[b, k*128+p]
    embT_l = np.ascontiguousarray(
        emb.T.reshape(KCH, P, B).transpose(1, 0, 2)
    ).astype(bf16)
    # emb_n[p, i, d] = emb[i*128+p, d]
    emb_n = np.ascontiguousarray(
        emb.reshape(NB, P, D).transpose(1, 0, 2)
    ).astype(bf16)

    bidx = np.arange(B)
    p_of_b = bidx % P
    i_of_b = bidx // P

    in_maps = []
    for c in range(NCORES):
        lo = c * CS
        wsh = w[lo : lo + CS]
        # wt_l[cw, p, k, cl] = wsh[cw*500+cl, k*128+p]
        wt_l = np.ascontiguousarray(
            wsh.T.reshape(KCH, P, NCW, CW).transpose(2, 1, 0, 3)
        ).astype(bf16)
        local = lab - lo
        in_shard = (local >= 0) & (local < CS)
        gidx = np.full((P, NB), CS, dtype=np.int32)  # CS -> OOB, skipped
        gidx[p_of_b, i_of_b] = np.where(in_shard, local, CS).astype(np.int32)
        in_maps.append(
            {
                "embT_l": embT_l,
                "wt_l": wt_l,
                "emb_n": emb_n,
                "w_nat": np.ascontiguousarray(wsh).astype(bf16),
                "gidx": gidx,
            }
        )
    return in_maps


def assemble_output(results, labels):
    """Host-side reassembly: window-major device blocks -> (B, C) f32,
    then overlay the corrected target logits from the owning shard."""
    lab = np.asarray(labels).astype(np.int64)
    out = np.empty((B, C), dtype=np.float32)
    for c in range(NCORES):
        blk = np.asarray(results[c]["out_dev"]).astype(np.float32)
        # blk[cw, p, i, cl] -> out[i*128+p, c*CS + cw*500 + cl]
        out[:, c * CS : (c + 1) * CS] = blk.transpose(2, 1, 0, 3).reshape(B, CS)
    tfv = np.stack(
        [np.asarray(results[c]["tfix"]).astype(np.float32) for c in range(NCORES)]
    )  # (NCORES, P, NB)
    core_of = lab // CS
    bidx = np.arange(B)
    out[bidx, lab] = tfv[core_of, bidx % P, bidx // P]
    return out


_CACHED_NC = None


def _get_graph():
    global _CACHED_NC
    if _CACHED_NC is None:
        _CACHED_NC = build_graph()
    return _CACHED_NC


def kernel(embeddings, labels, weight):
    from concourse.bass_utils import run_bass_kernel_spmd

    nc = _get_graph()
    in_maps = make_in_maps(embeddings, labels, weight)
    res = run_bass_kernel_spmd(nc, in_maps, core_ids=list(range(NCORES)))
    return assemble_output(res.results, labels)


if __name__ == "__main__":
    nc = build_graph()
    print("graph built ok")


# revision 8
# speedup vs baseline: 1.0795x; 1.0713x over previous
"""ArcFace head on 8 TRN2 NeuronCores (Bass/Tile).

Model-parallel over classes: each of the 8 cores owns a 12500-class slice
of the 100000-class weight matrix and computes its (1024 x 12500) slice of
the logits; the host reassembles slices along the class dim.

v2 design (vs v1): raw bf16 weights feed TensorE straight from DMA; the
1/||w_c|| and 64/||e_b|| normalizations are folded into the PSUM->SBUF
drain (scalar_tensor_tensor: psum * ebn[p] * wnb). Weight norms are
computed per class window from squares folded on DVE and reduced across
partitions (gpsimd partition_all_reduce or a ones-column matmul). The
ArcFace margin is computed early from gathered label rows (exact f32 cos)
into a tiny tfix output that the host overlays onto the final array - no
scatter into the big output, so no ordering tail. DRAM layouts are
window-major so every DMA moves contiguous multi-KB runs per partition.
"""

import math

import ml_dtypes
import numpy as np

import concourse.bacc as bacc
import concourse.bass as bass
import concourse.bass_isa as bass_isa
import concourse.mybir as mybir
import concourse.tile as tile

# Problem constants (hardcoded per harness rules).
B = 1024  # batch
D = 512  # embedding dim
C = 100000  # num classes
NCORES = 8
CS = C // NCORES  # classes per core = 12500
P = 128  # partitions
KCH = D // P  # contraction chunks = 4
NB = B // P  # batch tiles = 8
CW = 500  # class window (<=512 psum bank, divides 12500)
NCW = CS // CW  # 25 class windows

SCALE = 64.0
MARGIN = 0.5
COS_M = math.cos(MARGIN)
SIN_M = math.sin(MARGIN)
TH = math.cos(math.pi - MARGIN)
MM = math.sin(math.pi - MARGIN) * MARGIN

F32 = mybir.dt.float32
BF16 = mybir.dt.bfloat16
I32 = mybir.dt.int32
U8 = mybir.dt.uint8

NORM_MODE = "mm"  # "gpsimd" (partition_all_reduce) or "mm" (ones matmul)
PREP_LEAD = 3  # windows of weight-prep lead over the matmul consumer


def build_graph():
    nc = bacc.Bacc(
        "TRN2",
        target_bir_lowering=False,
        debug=False,
        num_devices=NCORES,
    )

    embT_l = nc.declare_dram_parameter("embT_l", [P, KCH, B], BF16, isOutput=False)
    wt_l = nc.declare_dram_parameter("wt_l", [NCW, P, KCH, CW], BF16, isOutput=False)
    emb_n = nc.declare_dram_parameter("emb_n", [P, NB, D], BF16, isOutput=False)
    w_nat = nc.declare_dram_parameter("w_nat", [CS, D], BF16, isOutput=False)
    gidx = nc.declare_dram_parameter("gidx", [P, NB], I32, isOutput=False)
    out_dev = nc.declare_dram_parameter(
        "out_dev", [NCW, P, NB, CW], BF16, isOutput=True
    )
    tfix = nc.declare_dram_parameter("tfix", [P, NB], BF16, isOutput=True)

    ADD = mybir.AluOpType.add
    MUL = mybir.AluOpType.mult

    with tile.TileContext(nc) as tc:
        with (
            tc.tile_pool(name="const", bufs=1) as constp,
            tc.tile_pool(name="embp", bufs=1) as embp,
            tc.tile_pool(name="wstage", bufs=4) as wstage,
            tc.tile_pool(name="w2p", bufs=2) as w2p,
            tc.tile_pool(name="wnbp", bufs=2) as wnbp,
            tc.tile_pool(name="wntp", bufs=PREP_LEAD + 1) as wntp,
            tc.tile_pool(name="nsqp", bufs=2) as nsqp,
            tc.tile_pool(name="ostripe", bufs=3) as ostripep,
            tc.tile_pool(name="marg", bufs=1) as margp,
            tc.tile_pool(name="ps_main", bufs=3, space="PSUM") as ps_main,
            tc.tile_pool(name="ps_small", bufs=2, space="PSUM") as ps_small,
        ):
            ones_col_bf = constp.tile([P, 1], BF16, tag="ones_col")
            nc.vector.memset(ones_col_bf[:], 1.0)

            # ---------- stationary inputs
            embT_t = embp.tile([P, KCH, B], BF16, tag="embT_t")
            nc.sync.dma_start(out=embT_t[:], in_=embT_l[:])
            emb_t = margp.tile([P, NB, D], BF16, tag="emb_t")
            nc.scalar.dma_start(out=emb_t[:], in_=emb_n[:])
            gidx_t = margp.tile([P, NB], I32, tag="gidx_t")
            nc.scalar.dma_start(out=gidx_t[:], in_=gidx[:])

            # ---------- per-window weight prep: wt DMA, then fold 1/||w_c||
            # into the weights themselves (wnt) so the PSUM drain only needs
            # the per-partition ebn scale (splittable across Act and DVE).
            def prep(cw):
                wt_f = wstage.tile([P, KCH, CW], BF16, tag="wt_f")
                nc.sync.dma_start(out=wt_f[:], in_=wt_l[cw])
                w2 = w2p.tile([P, KCH, CW], BF16, tag="w2")
                nc.scalar.square(w2[:], wt_f[:])
                w2b = w2p.tile([P, 2, CW], BF16, tag="w2b")
                nc.vector.tensor_add(w2b[:, 0, :], w2[:, 0, :], w2[:, 1, :])
                nc.vector.tensor_add(w2b[:, 1, :], w2[:, 2, :], w2[:, 3, :])
                w2s = w2p.tile([P, CW], BF16, tag="w2s")
                nc.vector.tensor_add(w2s[:], w2b[:, 0, :], w2b[:, 1, :])
                pn = ps_small.tile([1, 512], F32, tag="pn")
                nc.tensor.matmul(
                    pn[:, :CW],
                    lhsT=ones_col_bf[:],
                    rhs=w2s[:],
                    start=True,
                    stop=True,
                )
                rn = nsqp.tile([1, CW], F32, tag="rn")
                nc.scalar.sqrt(rn[:], pn[:, :CW])
                rrec = nsqp.tile([1, CW], F32, tag="rrec")
                nc.vector.reciprocal_approx_fast(rrec[:], rn[:])
                rrecb = nsqp.tile([1, CW], BF16, tag="rrecb")
                nc.scalar.copy(rrecb[:], rrec[:])
                wnb = wnbp.tile([P, CW], BF16, tag="wnb")
                nc.gpsimd.partition_broadcast(wnb[:], rrecb[:])
                wnt = wntp.tile([P, KCH, CW], BF16, tag="wnt")
                nc.vector.tensor_mul(
                    wnt[:],
                    wt_f[:],
                    wnb[:, None, :].to_broadcast([P, KCH, CW]),
                )
                return wnt

            win_q = [prep(cw) for cw in range(PREP_LEAD)]

            # ---------- embedding norms: ebn[p, i] = 64 / ||e_b||, b = i*128+p
            junk_e = margp.tile([P, D], BF16, tag="junk_e")
            en2 = margp.tile([P, NB], F32, tag="en2")
            for i in range(NB):
                nc.scalar.activation(
                    junk_e[:],
                    emb_t[:, i, :],
                    mybir.ActivationFunctionType.Square,
                    accum_out=en2[:, i : i + 1],
                )
            en_s = margp.tile([P, NB], F32, tag="en_s")
            nc.scalar.sqrt(en_s[:], en2[:])
            ebn_r = margp.tile([P, NB], F32, tag="ebn_r")
            ebn_scr = margp.tile([P, NB], F32, tag="ebn_scr")
            nc.vector.reciprocal_approx_accurate(ebn_r[:], en_s[:], ebn_scr[:])
            ebn = margp.tile([P, NB], F32, tag="ebn")
            nc.vector.tensor_scalar_mul(ebn[:], ebn_r[:], SCALE)

            # ---------- margin thunks (drained 2 per window inside the loop):
            # exact f32 target cos from gathered label rows -> tfix output;
            # the host overlays tfix, so nothing waits on the big output.
            wg = margp.tile([P, NB, D], BF16, tag="wg")
            junk_g = margp.tile([P, D], BF16, tag="junk_g")
            junk_d = margp.tile([P, D], BF16, tag="junk_d")
            gn2 = margp.tile([P, NB], F32, tag="gn2")
            dot = margp.tile([P, NB], F32, tag="dot")
            den = margp.tile([P, NB], F32, tag="den")
            rden = margp.tile([P, NB], F32, tag="rden")
            rscr = margp.tile([P, NB], F32, tag="rscr")
            cost = margp.tile([P, NB], F32, tag="cost")
            sint = margp.tile([P, NB], F32, tag="sint")
            cosm = margp.tile([P, NB], F32, tag="cosm")
            alt = margp.tile([P, NB], F32, tag="alt")
            mask = margp.tile([P, NB], U8, tag="mask")
            yv = margp.tile([P, NB], F32, tag="yv")
            tfix_t = margp.tile([P, NB], BF16, tag="tfix_t")

            margin_ops = []
            for i in range(NB):
                margin_ops.append(
                    lambda i=i: nc.gpsimd.indirect_dma_start(
                        out=wg[:, i, :],
                        out_offset=None,
                        in_=w_nat[:],
                        in_offset=bass.IndirectOffsetOnAxis(
                            ap=gidx_t[:, i : i + 1], axis=0
                        ),
                        bounds_check=CS - 1,
                        oob_is_err=False,
                    )
                )
                margin_ops.append(
                    lambda i=i: nc.scalar.activation(
                        junk_g[:],
                        wg[:, i, :],
                        mybir.ActivationFunctionType.Square,
                        accum_out=gn2[:, i : i + 1],
                    )
                )
            for i in range(NB):
                margin_ops.append(
                    lambda i=i: nc.vector.scalar_tensor_tensor(
                        out=junk_d[:],
                        in0=emb_t[:, i, :],
                        scalar=1.0,
                        in1=wg[:, i, :],
                        op0=MUL,
                        op1=MUL,
                        accum_out=dot[:, i : i + 1],
                    )
                )
            margin_ops += [
                # cos = dot / max(||e|| * ||w_l||, eps)
                lambda: nc.vector.tensor_mul(den[:], en2[:], gn2[:]),
                lambda: nc.scalar.sqrt(den[:], den[:]),
                lambda: nc.vector.tensor_scalar_max(den[:], den[:], 1e-12),
                lambda: nc.vector.reciprocal_approx_accurate(
                    rden[:], den[:], rscr[:]
                ),
                lambda: nc.vector.tensor_mul(cost[:], dot[:], rden[:]),
                # clip to +-(1 - 1e-7)
                lambda: nc.vector.tensor_scalar(
                    out=cost[:],
                    in0=cost[:],
                    scalar1=1.0 - 1e-7,
                    scalar2=-(1.0 - 1e-7),
                    op0=mybir.AluOpType.min,
                    op1=mybir.AluOpType.max,
                ),
                # sin = sqrt(max(0, 1 - cos^2))
                lambda: nc.vector.tensor_mul(sint[:], cost[:], cost[:]),
                lambda: nc.vector.tensor_scalar(
                    out=sint[:],
                    in0=sint[:],
                    scalar1=-1.0,
                    scalar2=1.0,
                    op0=MUL,
                    op1=ADD,
                ),
                lambda: nc.vector.tensor_scalar_max(sint[:], sint[:], 0.0),
                lambda: nc.scalar.sqrt(sint[:], sint[:]),
                # cos(t+m) = cos*COS_M - sin*SIN_M ; else: cos - MM
                lambda: nc.vector.tensor_scalar_mul(cosm[:], sint[:], -SIN_M),
                lambda: nc.vector.scalar_tensor_tensor(
                    out=cosm[:],
                    in0=cost[:],
                    scalar=COS_M,
                    in1=cosm[:],
                    op0=MUL,
                    op1=ADD,
                ),
                lambda: nc.vector.tensor_scalar_add(alt[:], cost[:], -MM),
                lambda: nc.vector.tensor_single_scalar(
                    mask[:], cost[:], TH, mybir.AluOpType.is_gt
                ),
                lambda: nc.vector.select(yv[:], mask[:], cosm[:], alt[:]),
                lambda: nc.vector.tensor_scalar_mul(tfix_t[:], yv[:], SCALE),
                lambda: nc.scalar.dma_start(out=tfix[:], in_=tfix_t[:]),
            ]

            # ---------- main loop: 25 windows x (32 matmuls + 8 drains + DMA)
            for cw in range(NCW):
                if cw + PREP_LEAD < NCW:
                    win_q.append(prep(cw + PREP_LEAD))
                wnt_cur = win_q.pop(0)
                ostripe = ostripep.tile([P, NB, CW], BF16, tag="ostripe")
                for half in range(NB // 2):
                    po2 = ps_main.tile([P, 2, 512], F32, tag="ps_main")
                    for j in range(2):
                        bt = half * 2 + j
                        for k in range(KCH):
                            nc.tensor.matmul(
                                po2[:, j, :CW],
                                lhsT=embT_t[:, k, bt * P : (bt + 1) * P],
                                rhs=wnt_cur[:, k, :],
                                start=(k == 0),
                                stop=(k == KCH - 1),
                            )
                    # drain = psum * ebn[p] (weights already carry 1/||w||);
                    # alternate engines so Act and DVE each take half
                    for j in range(2):
                        bt = half * 2 + j
                        if half % 2 == 0:
                            nc.scalar.mul(
                                ostripe[:, bt, :],
                                po2[:, j, :CW],
                                ebn[:, bt : bt + 1],
                            )
                        else:
                            nc.vector.tensor_scalar_mul(
                                ostripe[:, bt, :],
                                po2[:, j, :CW],
                                ebn[:, bt : bt + 1],
                            )
                nc.gpsimd.dma_start(out=out_dev[cw], in_=ostripe[:])
                for _ in range(2):
                    if margin_ops:
                        margin_ops.pop(0)()
            while margin_ops:
                margin_ops.pop(0)()

    nc.compile()
    return nc


def make_in_maps(embeddings, labels, weight):
    """Shard + lay out the inputs for the 8 cores (host-side layout prep)."""
    emb = np.ascontiguousarray(embeddings, dtype=np.float32)
    lab = np.asarray(labels).astype(np.int64)
    w = np.asarray(weight, dtype=np.float32)

    bf16 = ml_dtypes.bfloat16
    # embT_l[p, k, b] = emb[b, k*128+p]
    embT_l = np.ascontiguousarray(
        emb.T.reshape(KCH, P, B).transpose(1, 0, 2)
    ).astype(bf16)
    # emb_n[p, i, d] = emb[i*128+p, d]
    emb_n = np.ascontiguousarray(
        emb.reshape(NB, P, D).transpose(1, 0, 2)
    ).astype(bf16)

    bidx = np.arange(B)
    p_of_b = bidx % P
    i_of_b = bidx // P

    in_maps = []
    for c in range(NCORES):
        lo = c * CS
        wsh = w[lo : lo + CS]
        # wt_l[cw, p, k, cl] = wsh[cw*500+cl, k*128+p]
        wt_l = np.ascontiguousarray(
            wsh.T.reshape(KCH, P, NCW, CW).transpose(2, 1, 0, 3)
        ).astype(bf16)
        local = lab - lo
        in_shard = (local >= 0) & (local < CS)
        gidx = np.full((P, NB), CS, dtype=np.int32)  # CS -> OOB, skipped
        gidx[p_of_b, i_of_b] = np.where(in_shard, local, CS).astype(np.int32)
        in_maps.append(
            {
                "embT_l": embT_l,
                "wt_l": wt_l,
                "emb_n": emb_n,
                "w_nat": np.ascontiguousarray(wsh).astype(bf16),
                "gidx": gidx,
            }
        )
    return in_maps


def assemble_output(results, labels):
    """Host-side reassembly: window-major device blocks -> (B, C) f32,
    then overlay the corrected target logits from the owning shard."""
    lab = np.asarray(labels).astype(np.int64)
    out = np.empty((B, C), dtype=np.float32)
    for c in range(NCORES):
        blk = np.asarray(results[c]["out_dev"]).astype(np.float32)
        # blk[cw, p, i, cl] -> out[i*128+p, c*CS + cw*500 + cl]
        out[:, c * CS : (c + 1) * CS] = blk.transpose(2, 1, 0, 3).reshape(B, CS)
    tfv = np.stack(
        [np.asarray(results[c]["tfix"]).astype(np.float32) for c in range(NCORES)]
    )  # (NCORES, P, NB)
    core_of = lab // CS
    bidx = np.arange(B)
    out[bidx, lab] = tfv[core_of, bidx % P, bidx // P]
    return out


_CACHED_NC = None


def _get_graph():
    global _CACHED_NC
    if _CACHED_NC is None:
        _CACHED_NC = build_graph()
    return _CACHED_NC


def kernel(embeddings, labels, weight):
    from concourse.bass_utils import run_bass_kernel_spmd

    nc = _get_graph()
    in_maps = make_in_maps(embeddings, labels, weight)
    res = run_bass_kernel_spmd(nc, in_maps, core_ids=list(range(NCORES)))
    return assemble_output(res.results, labels)


if __name__ == "__main__":
    nc = build_graph()
    print("graph built ok")


# revision 12
# speedup vs baseline: 1.1398x; 1.0558x over previous
"""ArcFace head on 8 TRN2 NeuronCores (Bass/Tile).

Model-parallel over classes: each of the 8 cores owns a 12500-class slice
of the 100000-class weight matrix and computes its (1024 x 12500) slice of
the logits; the host reassembles slices along the class dim.

v4 schedule: two-stage weight prep. Stage A (wt DMA + square + k-folds)
leads the consumer by 3 windows; stage B (ones-matmul norm reduce + sqrt +
reciprocal + broadcast + weight scale) is emitted in the middle of the
matmul stream two windows ahead, so a lagging prep never head-of-line
blocks TensorE. The first 3 windows consume RAW bf16 weights straight from
DMA (drains apply both 1/||w_c|| and 64/||e_b||) so matmuls start ~3us in;
later windows use pre-scaled weights so the drain is a per-partition scale
that splits across Act and DVE. The ArcFace margin is computed from
gathered label rows into a tiny tfix output the host overlays - nothing
orders against the big output write.
"""

import math

import ml_dtypes
import numpy as np

import concourse.bacc as bacc
import concourse.bass as bass
import concourse.mybir as mybir
import concourse.tile as tile

# Problem constants (hardcoded per harness rules).
B = 1024  # batch
D = 512  # embedding dim
C = 100000  # num classes
NCORES = 8
CS = C // NCORES  # classes per core = 12500
P = 128  # partitions
KCH = D // P  # contraction chunks = 4
NB = B // P  # batch tiles = 8
CW = 500  # class window (<=512 psum bank, divides 12500)
NCW = CS // CW  # 25 class windows

SCALE = 64.0
MARGIN = 0.5
COS_M = math.cos(MARGIN)
SIN_M = math.sin(MARGIN)
TH = math.cos(math.pi - MARGIN)
MM = math.sin(math.pi - MARGIN) * MARGIN

F32 = mybir.dt.float32
BF16 = mybir.dt.bfloat16
I32 = mybir.dt.int32
U8 = mybir.dt.uint8

NRAW = 3  # leading windows that consume raw weights (fused drain)
A_LEAD = 3  # stage-A (DMA+square+folds) lead over consumer
B_LEAD = 2  # stage-B (norm chain) lead over consumer


def build_graph():
    nc = bacc.Bacc(
        "TRN2",
        target_bir_lowering=False,
        debug=False,
        num_devices=NCORES,
    )

    embT_l = nc.declare_dram_parameter("embT_l", [P, KCH, B], BF16, isOutput=False)
    wt_l = nc.declare_dram_parameter("wt_l", [NCW, P, KCH, CW], BF16, isOutput=False)
    emb_n = nc.declare_dram_parameter("emb_n", [P, NB, D], BF16, isOutput=False)
    w_nat = nc.declare_dram_parameter("w_nat", [CS, D], BF16, isOutput=False)
    gidx = nc.declare_dram_parameter("gidx", [P, NB], I32, isOutput=False)
    out_dev = nc.declare_dram_parameter(
        "out_dev", [NCW, P, NB, CW], BF16, isOutput=True
    )
    tfix = nc.declare_dram_parameter("tfix", [P, NB], BF16, isOutput=True)

    ADD = mybir.AluOpType.add
    MUL = mybir.AluOpType.mult
    SQ = mybir.ActivationFunctionType.Square

    with tile.TileContext(nc) as tc:
        with (
            tc.tile_pool(name="const", bufs=1) as constp,
            tc.tile_pool(name="embp", bufs=1) as embp,
            tc.tile_pool(name="wstage", bufs=6) as wstage,
            tc.tile_pool(name="w2p", bufs=2) as w2p,
            tc.tile_pool(name="w2sp", bufs=3) as w2sp,
            tc.tile_pool(name="wnbfp", bufs=2) as wnbfp,
            tc.tile_pool(name="wnbp", bufs=2) as wnbp,
            tc.tile_pool(name="wntp", bufs=3) as wntp,
            tc.tile_pool(name="nsqp", bufs=2) as nsqp,
            tc.tile_pool(name="ostripe", bufs=3) as ostripep,
            tc.tile_pool(name="marg", bufs=1) as margp,
            tc.tile_pool(name="ps_main", bufs=3, space="PSUM") as ps_main,
            tc.tile_pool(name="ps_small", bufs=2, space="PSUM") as ps_small,
        ):
            ones_col_bf = constp.tile([P, 1], BF16, tag="ones_col")
            nc.vector.memset(ones_col_bf[:], 1.0)

            # ---------- stationary inputs, split across queues for fast start
            embT_t = embp.tile([P, KCH, B], BF16, tag="embT_t")
            emb_t = margp.tile([P, NB, D], BF16, tag="emb_t")
            gidx_t = margp.tile([P, NB], I32, tag="gidx_t")

            # stage A: wt DMA + square + k-fold (leads by A_LEAD windows)
            wt_tiles = {}

            def stage_a(cw):
                wt_f = wstage.tile([P, KCH, CW], BF16, tag="wt_f")
                nc.sync.dma_start(out=wt_f[:], in_=wt_l[cw])
                w2 = w2p.tile([P, KCH, CW], BF16, tag="w2")
                nc.scalar.square(w2[:], wt_f[:])
                w2b = w2sp.tile([P, 2, CW], BF16, tag="w2b")
                nc.vector.tensor_add(w2b[:, 0, :], w2[:, 0, :], w2[:, 1, :])
                nc.vector.tensor_add(w2b[:, 1, :], w2[:, 2, :], w2[:, 3, :])
                w2s = w2sp.tile([P, CW], BF16, tag="w2s")
                nc.vector.tensor_add(w2s[:], w2b[:, 0, :], w2b[:, 1, :])
                wt_tiles[cw] = (wt_f, w2s)

            # stage B: norm reduce on TensorE + sqrt/recip + broadcast (+
            # weight pre-scale for cw >= NRAW). Returns drain operands.
            drain_ops = {}

            def stage_b(cw):
                wt_f, w2s = wt_tiles.pop(cw)
                pn = ps_small.tile([1, 512], F32, tag="pn")
                nc.tensor.matmul(
                    pn[:, :CW], lhsT=ones_col_bf[:], rhs=w2s[:],
                    start=True, stop=True,
                )
                rn = nsqp.tile([1, CW], F32, tag="rn")
                nc.scalar.sqrt(rn[:], pn[:, :CW])
                rrec = nsqp.tile([1, CW], F32, tag="rrec")
                nc.vector.reciprocal_approx_fast(rrec[:], rn[:])
                if cw < NRAW:
                    wnb_f = wnbfp.tile([P, CW], F32, tag="wnb_f")
                    nc.gpsimd.partition_broadcast(wnb_f[:], rrec[:])
                    drain_ops[cw] = ("raw", wt_f, wnb_f)
                else:
                    rrecb = nsqp.tile([1, CW], BF16, tag="rrecb")
                    nc.scalar.copy(rrecb[:], rrec[:])
                    wnb = wnbp.tile([P, CW], BF16, tag="wnb")
                    nc.gpsimd.partition_broadcast(wnb[:], rrecb[:])
                    wnt = wntp.tile([P, KCH, CW], BF16, tag="wnt")
                    nc.vector.tensor_mul(
                        wnt[:],
                        wt_f[:],
                        wnb[:, None, :].to_broadcast([P, KCH, CW]),
                    )
                    drain_ops[cw] = ("scaled", wnt, None)

            # queue layout: interleave first wt loads with the embT halves
            stage_a(0)  # emits wt(0) DMA first on sync queue
            nc.sync.dma_start(out=embT_t[:, :, :512], in_=embT_l[:, :, :512])
            stage_a(1)
            nc.sync.dma_start(out=embT_t[:, :, 512:], in_=embT_l[:, :, 512:])
            stage_a(2)
            nc.scalar.dma_start(out=emb_t[:, :4, :], in_=emb_n[:, :4, :])
            nc.scalar.dma_start(out=emb_t[:, 4:, :], in_=emb_n[:, 4:, :])
            nc.scalar.dma_start(out=gidx_t[:], in_=gidx[:])

            # ---------- embedding norms: ebn[p, i] = 64/||e_b||, b = i*128+p
            # split across Act (i 0-3) and DVE (i 4-7) to finish by ~6.5us
            junk_e = margp.tile([P, D], BF16, tag="junk_e")
            junk_v = margp.tile([P, D], BF16, tag="junk_v")
            en2 = margp.tile([P, NB], F32, tag="en2")
            for i in range(4):
                nc.scalar.activation(
                    junk_e[:], emb_t[:, i, :], SQ,
                    accum_out=en2[:, i : i + 1],
                )
            for i in range(4, NB):
                nc.vector.scalar_tensor_tensor(
                    out=junk_v[:],
                    in0=emb_t[:, i, :],
                    scalar=1.0,
                    in1=emb_t[:, i, :],
                    op0=MUL,
                    op1=MUL,
                    accum_out=en2[:, i : i + 1],
                )
            en_s = margp.tile([P, NB], F32, tag="en_s")
            nc.scalar.sqrt(en_s[:], en2[:])
            ebn_r = margp.tile([P, NB], F32, tag="ebn_r")
            ebn_scr = margp.tile([P, NB], F32, tag="ebn_scr")
            nc.vector.reciprocal_approx_accurate(ebn_r[:], en_s[:], ebn_scr[:])
            ebn = margp.tile([P, NB], F32, tag="ebn")
            nc.vector.tensor_scalar_mul(ebn[:], ebn_r[:], SCALE)

            # ---------- margin thunks (drained 2 per window inside the loop):
            # exact f32 target cos from gathered label rows -> tfix output;
            # the host overlays tfix, so nothing waits on the big output.
            wg = margp.tile([P, NB, D], BF16, tag="wg")
            junk_g = margp.tile([P, D], BF16, tag="junk_g")
            junk_d = margp.tile([P, D], BF16, tag="junk_d")
            gn2 = margp.tile([P, NB], F32, tag="gn2")
            dot = margp.tile([P, NB], F32, tag="dot")
            den = margp.tile([P, NB], F32, tag="den")
            rden = margp.tile([P, NB], F32, tag="rden")
            rscr = margp.tile([P, NB], F32, tag="rscr")
            cost = margp.tile([P, NB], F32, tag="cost")
            sint = margp.tile([P, NB], F32, tag="sint")
            cosm = margp.tile([P, NB], F32, tag="cosm")
            alt = margp.tile([P, NB], F32, tag="alt")
            mask = margp.tile([P, NB], U8, tag="mask")
            yv = margp.tile([P, NB], F32, tag="yv")
            tfix_t = margp.tile([P, NB], BF16, tag="tfix_t")

            margin_ops = []
            for i in range(NB):
                margin_ops.append(
                    lambda i=i: nc.gpsimd.indirect_dma_start(
                        out=wg[:, i, :],
                        out_offset=None,
                        in_=w_nat[:],
                        in_offset=bass.IndirectOffsetOnAxis(
                            ap=gidx_t[:, i : i + 1], axis=0
                        ),
                        bounds_check=CS - 1,
                        oob_is_err=False,
                    )
                )
                margin_ops.append(
                    lambda i=i: nc.scalar.activation(
                        junk_g[:], wg[:, i, :], SQ,
                        accum_out=gn2[:, i : i + 1],
                    )
                )
            for i in range(NB):
                margin_ops.append(
                    lambda i=i: nc.vector.scalar_tensor_tensor(
                        out=junk_d[:],
                        in0=emb_t[:, i, :],
                        scalar=1.0,
                        in1=wg[:, i, :],
                        op0=MUL,
                        op1=MUL,
                        accum_out=dot[:, i : i + 1],
                    )
                )
            margin_ops += [
                # cos = dot / max(||e|| * ||w_l||, eps)
                lambda: nc.vector.tensor_mul(den[:], en2[:], gn2[:]),
                lambda: nc.scalar.sqrt(den[:], den[:]),
                lambda: nc.vector.tensor_scalar_max(den[:], den[:], 1e-12),
                lambda: nc.vector.reciprocal_approx_accurate(
                    rden[:], den[:], rscr[:]
                ),
                lambda: nc.vector.tensor_mul(cost[:], dot[:], rden[:]),
                # clip to +-(1 - 1e-7)
                lambda: nc.vector.tensor_scalar(
                    out=cost[:],
                    in0=cost[:],
                    scalar1=1.0 - 1e-7,
                    scalar2=-(1.0 - 1e-7),
                    op0=mybir.AluOpType.min,
                    op1=mybir.AluOpType.max,
                ),
                # sin = sqrt(max(0, 1 - cos^2))
                lambda: nc.vector.tensor_mul(sint[:], cost[:], cost[:]),
                lambda: nc.vector.tensor_scalar(
                    out=sint[:],
                    in0=sint[:],
                    scalar1=-1.0,
                    scalar2=1.0,
                    op0=MUL,
                    op1=ADD,
                ),
                lambda: nc.vector.tensor_scalar_max(sint[:], sint[:], 0.0),
                lambda: nc.scalar.sqrt(sint[:], sint[:]),
                # cos(t+m) = cos*COS_M - sin*SIN_M ; else: cos - MM
                lambda: nc.vector.tensor_scalar_mul(cosm[:], sint[:], -SIN_M),
                lambda: nc.vector.scalar_tensor_tensor(
                    out=cosm[:],
                    in0=cost[:],
                    scalar=COS_M,
                    in1=cosm[:],
                    op0=MUL,
                    op1=ADD,
                ),
                lambda: nc.vector.tensor_scalar_add(alt[:], cost[:], -MM),
                lambda: nc.vector.tensor_single_scalar(
                    mask[:], cost[:], TH, mybir.AluOpType.is_gt
                ),
                lambda: nc.vector.select(yv[:], mask[:], cosm[:], alt[:]),
                lambda: nc.vector.tensor_scalar_mul(tfix_t[:], yv[:], SCALE),
                lambda: nc.scalar.dma_start(out=tfix[:], in_=tfix_t[:]),
            ]

            # ---------- main loop: 25 windows x (32 matmuls + 8 drains + DMA)
            # stage-B emission points inside window 0: cw=0 after half 0,
            # cw=1 after half 2; from window 1 on: cw+B_LEAD after half 1.
            for cw in range(NCW):
                if cw + A_LEAD < NCW:
                    stage_a(cw + A_LEAD)
                ostripe = ostripep.tile([P, NB, CW], BF16, tag="ostripe")
                halves = []
                for half in range(NB // 2):
                    po2 = ps_main.tile([P, 2, 512], F32, tag="ps_main")
                    halves.append(po2)
                    # matmuls for this half
                    for j in range(2):
                        bt = half * 2 + j
                        for k in range(KCH):
                            src = (
                                drain_ops[cw][1]
                                if cw in drain_ops and drain_ops[cw][0] == "scaled"
                                else wt_tiles[cw][0]
                                if cw in wt_tiles
                                else drain_ops[cw][1]
                            )
                            nc.tensor.matmul(
                                po2[:, j, :CW],
                                lhsT=embT_t[:, k, bt * P : (bt + 1) * P],
                                rhs=src[:, k, :],
                                start=(k == 0),
                                stop=(k == KCH - 1),
                            )
                    # stage-B emission between matmul halves
                    if cw == 0 and half in (0, 1, 2):
                        stage_b(half)
                    elif cw >= 1 and half == 1 and cw + B_LEAD < NCW:
                        stage_b(cw + B_LEAD)
                    # drains for this half
                    mode, wsrc, wnb_f = drain_ops[cw]
                    for j in range(2):
                        bt = half * 2 + j
                        if mode == "raw":
                            nc.vector.scalar_tensor_tensor(
                                out=ostripe[:, bt, :],
                                in0=po2[:, j, :CW],
                                scalar=ebn[:, bt : bt + 1],
                                in1=wnb_f[:],
                                op0=MUL,
                                op1=MUL,
                            )
                        elif half % 2 == 0:
                            nc.scalar.mul(
                                ostripe[:, bt, :],
                                po2[:, j, :CW],
                                ebn[:, bt : bt + 1],
                            )
                        else:
                            nc.vector.tensor_scalar_mul(
                                ostripe[:, bt, :],
                                po2[:, j, :CW],
                                ebn[:, bt : bt + 1],
                            )
                nc.gpsimd.dma_start(out=out_dev[cw], in_=ostripe[:])
                drain_ops.pop(cw, None)
                for _ in range(2):
                    if margin_ops:
                        margin_ops.pop(0)()
            while margin_ops:
                margin_ops.pop(0)()

    nc.compile()
    return nc


def make_in_maps(embeddings, labels, weight):
    """Shard + lay out the inputs for the 8 cores (host-side layout prep)."""
    emb = np.ascontiguousarray(embeddings, dtype=np.float32)
    lab = np.asarray(labels).astype(np.int64)
    w = np.asarray(weight, dtype=np.float32)

    bf16 = ml_dtypes.bfloat16
    # embT_l[p, k, b] = emb[b, k*128+p]
    embT_l = np.ascontiguousarray(
        emb.T.reshape(KCH, P, B).transpose(1, 0, 2)
    ).astype(bf16)
    # emb_n[p, i, d] = emb[i*128+p, d]
    emb_n = np.ascontiguousarray(
        emb.reshape(NB, P, D).transpose(1, 0, 2)
    ).astype(bf16)

    bidx = np.arange(B)
    p_of_b = bidx % P
    i_of_b = bidx // P

    in_maps = []
    for c in range(NCORES):
        lo = c * CS
        wsh = w[lo : lo + CS]
        # wt_l[cw, p, k, cl] = wsh[cw*500+cl, k*128+p]
        wt_l = np.ascontiguousarray(
            wsh.T.reshape(KCH, P, NCW, CW).transpose(2, 1, 0, 3)
        ).astype(bf16)
        local = lab - lo
        in_shard = (local >= 0) & (local < CS)
        gidx = np.full((P, NB), CS, dtype=np.int32)  # CS -> OOB, skipped
        gidx[p_of_b, i_of_b] = np.where(in_shard, local, CS).astype(np.int32)
        in_maps.append(
            {
                "embT_l": embT_l,
                "wt_l": wt_l,
                "emb_n": emb_n,
                "w_nat": np.ascontiguousarray(wsh).astype(bf16),
                "gidx": gidx,
            }
        )
    return in_maps


def assemble_output(results, labels):
    """Host-side reassembly: window-major device blocks -> (B, C) f32,
    then overlay the corrected target logits from the owning shard."""
    lab = np.asarray(labels).astype(np.int64)
    out = np.empty((B, C), dtype=np.float32)
    for c in range(NCORES):
        blk = np.asarray(results[c]["out_dev"]).astype(np.float32)
        # blk[cw, p, i, cl] -> out[i*128+p, c*CS + cw*500 + cl]
        out[:, c * CS : (c + 1) * CS] = blk.transpose(2, 1, 0, 3).reshape(B, CS)
    tfv = np.stack(
        [np.asarray(results[c]["tfix"]).astype(np.float32) for c in range(NCORES)]
    )  # (NCORES, P, NB)
    core_of = lab // CS
    bidx = np.arange(B)
    out[bidx, lab] = tfv[core_of, bidx % P, bidx // P]
    return out


_CACHED_NC = None


def _get_graph():
    global _CACHED_NC
    if _CACHED_NC is None:
        _CACHED_NC = build_graph()
    return _CACHED_NC


def kernel(embeddings, labels, weight):
    from concourse.bass_utils import run_bass_kernel_spmd

    nc = _get_graph()
    in_maps = make_in_maps(embeddings, labels, weight)
    res = run_bass_kernel_spmd(nc, in_maps, core_ids=list(range(NCORES)))
    return assemble_output(res.results, labels)


if __name__ == "__main__":
    nc = build_graph()
    print("graph built ok")
